# revision 19
# baseline (speedup 1.0000x reference)
"""Causal self-attention (B=8, T=1024, C=1024, H=16) on 8 TRN2 NeuronCores.

Sharding: pure data-parallel over batch - core b computes batch element b
with fully replicated weights (B == n_cores, so no collectives needed).

v2 vs baseline:
  - x / Wqkv / Wproj are cast to bf16 on the HOST, so no on-device weight
    casts and half the weight DMA traffic.
  - x is transposed by the DMA xbar (dma_start_transpose) straight from
    DRAM into SBUF - no PE transposes, no PSUM evacuation copies.
  - QKV / V / proj matmul loops are ordered so consecutive matmuls share
    the stationary operand (halves LDWEIGHTS traffic).
  - Score matmuls for the head pair are emitted interleaved with explicit
    tile_position (0,0)/(64,0) so the PE can run both 64-contraction
    matmuls concurrently in different row groups.
  - Y evacuation split across ACT+DVE; V/proj-bias evacuations on Pool;
    softmax reciprocal chain batched per head with a DMA reblock; the
    last pair uses a DVE-recip + PE-broadcast fast path to cut the tail.
"""

import numpy as np
import ml_dtypes

import concourse.tile as tile
from concourse import bacc, mybir
from concourse.bass_utils import run_bass_kernel_spmd
from concourse.masks import make_identity

f32 = mybir.dt.float32
bf16 = mybir.dt.bfloat16
AF = mybir.ActivationFunctionType
ALU = mybir.AluOpType

B, T, C, H, HD = 8, 1024, 1024, 16, 64
P = 128
NT = T // P  # 8 token tiles
NS = C // P  # 8 contraction subtiles
W = 66  # per-head stride in V_sb: [64 vals][1 ones][1 pad]


def _build():
    nc = bacc.Bacc(trn_type="TRN2")
    x_d = nc.dram_tensor("x", (T, C), bf16, kind="ExternalInput")
    wqkv_d = nc.dram_tensor("wqkv", (C, 3 * C), bf16, kind="ExternalInput")
    bqkv_d = nc.dram_tensor("bqkv", (3 * C,), f32, kind="ExternalInput")
    wproj_d = nc.dram_tensor("wproj", (C, C), bf16, kind="ExternalInput")
    bproj_d = nc.dram_tensor("bproj", (C,), f32, kind="ExternalInput")
    out_d = nc.dram_tensor("out", (T, C), f32, kind="ExternalOutput")

    with tile.TileContext(nc) as tc:
        with (
            tc.tile_pool(name="big", bufs=1) as big,
            tc.tile_pool(name="stage", bufs=3) as stage,
            tc.tile_pool(name="ptp", bufs=4) as ptp,
            tc.tile_pool(name="small", bufs=3) as small,
            tc.tile_pool(name="small1", bufs=1) as small1,
            tc.tile_pool(name="outp", bufs=3) as outp,
            tc.tile_pool(name="dramp", bufs=6, space="DRAM") as dramp,
            tc.tile_pool(name="pmm", bufs=2, space="PSUM") as pmm,
            tc.tile_pool(name="psp", bufs=2, space="PSUM") as psp,
            tc.tile_pool(name="pyp", bufs=1, space="PSUM") as pyp,
        ):
            ident = big.tile([P, P], bf16, tag="ident")
            make_identity(nc, ident)

            # ---------------- weights (bf16, no casts) ----------------
            # Q/K m-pair slices on the ACT ring so they overlap the x
            # stages on the sync ring; ACT's ring is idle at startup
            # (first exp is ~15us in) and these issues finish well before.
            # Ordered mp 0,4,1,5,... so each pair's Q (mp<4) and K (mp>=4)
            # slices arrive together.
            wqkv_sb = big.tile([P, NS, 3 * C], bf16, tag="wqkv")
            wq_r = wqkv_d[:, :].rearrange("(s p) i -> p s i", p=P)
            for mp in (0, 4, 1, 5, 2, 6, 3, 7):
                nc.scalar.dma_start(
                    wqkv_sb[:, :, mp * 256 : (mp + 1) * 256],
                    wq_r[:, :, mp * 256 : (mp + 1) * 256],
                )

            # ---------------- x: stage + PE transpose (warms the PE) ------
            xt_sb = big.tile([P, NS, T], bf16, tag="xt")
            x_r = x_d[:, :].rearrange("(i p) c -> p i c", p=P)
            for i in range(NT):
                xst = stage.tile([P, C], bf16, tag="stage", name=f"xst{i}")
                nc.sync.dma_start(xst, x_r[:, i, :])
                for jh in range(2):
                    tp = pmm.tile([P, 512], bf16, tag="pmm", name=f"xt{i}_{jh}")
                    for jj in range(4):
                        j = 4 * jh + jj
                        nc.tensor.transpose(
                            tp[:, jj * P : (jj + 1) * P],
                            xst[:, j * P : (j + 1) * P],
                            ident,
                        )
                    nc.vector.tensor_copy(
                        xt_sb[:, 4 * jh : 4 * jh + 4, i * P : (i + 1) * P],
                        tp.rearrange("p (j t) -> p j t", t=P),
                    )

            # V columns per s-row (2KB descriptors)
            for s in range(NS):
                nc.sync.dma_start(
                    wqkv_sb[:, s, 2 * C : 3 * C], wq_r[:, s, 2 * C : 3 * C]
                )

            # ---------------- constants ----------------
            # causal multiplicative mask for the transposed diagonal block:
            # cmask[k, q] = 1 if q >= k else 0
            cmask = big.tile([P, P], bf16, tag="cmask")
            nc.gpsimd.memset(cmask, 1.0)
            nc.gpsimd.affine_select(
                out=cmask,
                in_=cmask,
                compare_op=ALU.is_ge,
                fill=0.0,
                base=0,
                pattern=[[1, P]],
                channel_multiplier=-1,
            )
            # per-partition bias columns for the Q/K part of qkvT
            bqk_col = big.tile([P, 2 * C // P], f32, tag="bqk")
            nc.gpsimd.dma_start(bqk_col, bqkv_d[: 2 * C].rearrange("(o p) -> p o", p=P))
            # broadcast bias rows (per free-dim column) for V and proj
            bias_v = big.tile([P, C], f32, tag="bias_v")
            nc.gpsimd.dma_start(bias_v, bqkv_d[2 * C :][None, :].to_broadcast((P, C)))
            wproj_sb = big.tile([P, NS, C], bf16, tag="wproj")
            wp_r = wproj_d[:, :].rearrange("(s p) j -> p s j", p=P)

            # ---------------- Q/K^T tiles (interleaved with attention) ----
            qkt_sb = big.tile([P, 2 * C // P, T], bf16, tag="qkt")

            def emit_qk(m):
                # ch-inner so both matmuls share one LDWEIGHTS per s
                ps0 = pmm.tile([P, 512], f32, tag="pmm", name=f"qk{m}_0")
                ps1 = pmm.tile([P, 512], f32, tag="pmm", name=f"qk{m}_1")
                for s in range(NS):
                    for ch, ps in ((0, ps0), (1, ps1)):
                        nc.tensor.matmul(
                            ps,
                            wqkv_sb[:, s, m * P : (m + 1) * P],
                            xt_sb[:, s, ch * 512 : (ch + 1) * 512],
                            start=(s == 0),
                            stop=(s == NS - 1),
                        )
                nc.vector.tensor_scalar_add(
                    qkt_sb[:, m, 0:512], ps0, bqk_col[:, m : m + 1]
                )
                nc.vector.tensor_scalar_add(
                    qkt_sb[:, m, 512:T], ps1, bqk_col[:, m : m + 1]
                )

            # pair 0's Q/K first so ScalarE's exp pipeline starts early
            emit_qk(0)
            emit_qk(C // P)

            # ---------------- V (natural layout, ones-augmented) ----------
            v_sb = [
                big.tile([P, H * W], bf16, tag=f"v{i}", name=f"v{i}") for i in range(NT)
            ]

            def emit_v(i):
                v3 = v_sb[i].rearrange("p (h w) -> p h w", w=W)
                nc.gpsimd.memset(v3[:, :, HD : HD + 1], 1.0)
                ps0 = pmm.tile([P, 512], f32, tag="pmm", name=f"v{i}_0")
                ps1 = pmm.tile([P, 512], f32, tag="pmm", name=f"v{i}_1")
                for s in range(NS):
                    for ch, ps in ((0, ps0), (1, ps1)):
                        nc.tensor.matmul(
                            ps,
                            xt_sb[:, s, i * P : (i + 1) * P],
                            wqkv_sb[:, s, 2 * C + ch * 512 : 2 * C + (ch + 1) * 512],
                            start=(s == 0),
                            stop=(s == NS - 1),
                        )
                for ch, ps in ((0, ps0), (1, ps1)):
                    nc.vector.tensor_tensor(
                        v3[:, 8 * ch : 8 * ch + 8, 0:HD],
                        ps.rearrange("p (h d) -> p h d", d=HD),
                        bias_v[:, ch * 512 : (ch + 1) * 512].rearrange(
                            "p (h d) -> p h d", d=HD
                        ),
                        ALU.add,
                    )

            # ---------------- attention ----------------
            yt_sb = [
                big.tile([P, T], bf16, tag=f"yt{g}", name=f"yt{g}") for g in range(NT)
            ]

            def s_matmuls(sp0, sp1, kt_h0, qt_h0, kt_h1, qt_h1, kt):
                # interleave the two heads' matmuls so they occupy
                # different PE row groups concurrently
                q0 = kt * P
                if kt <= 3:
                    spans = [(q0, 512), (512, T)]
                else:
                    spans = [(q0, T)]
                for lo, hi in spans:
                    nc.tensor.matmul(
                        sp0[:, lo:hi],
                        kt_h0[:, q0 : q0 + P],
                        qt_h0[:, lo:hi],
                        start=True,
                        stop=True,
                        tile_position=(0, 0),
                    )
                    nc.tensor.matmul(
                        sp1[:, lo:hi],
                        kt_h1[:, q0 : q0 + P],
                        qt_h1[:, lo:hi],
                        start=True,
                        stop=True,
                        tile_position=(64, 0),
                    )

            def av_matmuls(ypA, ypB, pt_ap, vcols, kt, q_off):
                # ypA covers q columns [0,512), ypB [512,T); pt_ap covers
                # q columns [q_off, T); accumulate over kt
                q0 = kt * P
                lhsT_v = v_sb[kt][:, vcols : vcols + HD + 1]  # [128, 65]
                if kt <= 3:
                    nc.tensor.matmul(
                        ypA[0 : HD + 1, q0:512],
                        lhsT_v,
                        pt_ap[:, q0 - q_off : 512 - q_off],
                        start=(kt == 0),
                        stop=(kt == 3),
                    )
                    nc.tensor.matmul(
                        ypB[0 : HD + 1, 0:512],
                        lhsT_v,
                        pt_ap[:, 512 - q_off : T - q_off],
                        start=(kt == 0),
                        stop=(kt == NT - 1),
                    )
                else:
                    nc.tensor.matmul(
                        ypB[0 : HD + 1, q0 - 512 : 512],
                        lhsT_v,
                        pt_ap[:, q0 - q_off : T - q_off],
                        start=False,
                        stop=(kt == NT - 1),
                    )

            def evac_headA(ypA, h):
                # the A half (q cols 0:512) finishes accumulating at kt=3,
                # so it is evacuated early, off the critical path
                yu = small.tile([HD + 1, T], bf16, tag="yu", name=f"yu{h}")
                nc.scalar.copy(yu[:, 0:512], ypA[0 : HD + 1, 0:512])
                return yu

            def evac_headB(yu, ypB, h, s64_pair, half, fast):
                # the B half gates the psum recycle: split ACT/DVE quarters
                nc.scalar.copy(yu[:, 512:768], ypB[0 : HD + 1, 0:256])
                nc.vector.tensor_copy(yu[:, 768:T], ypB[0 : HD + 1, 256:512])
                # start the reciprocal DMA chain (reblock via DRAM); both
                # heads of a pair land in one s64 tile so one reciprocal
                # serves the pair
                dma = nc.sync.dma_start if fast else nc.gpsimd.dma_start
                scr = dramp.tile([T], bf16, tag="scr", name=f"scr{h}")
                dma(scr[None, :], yu[HD : HD + 1, :])
                e = T // HD
                dma(
                    s64_pair[:, half * e : (half + 1) * e],
                    scr.rearrange("(p e) -> p e", p=HD),
                )

            def norm_pair(state):
                # one reciprocal per pair + per-head broadcast + normalize;
                # emitted ~a pair later so the DVE never head-of-line blocks
                yu_on, h_on, yu_def, h_def, s64_pair, fast = state
                g = h_on // 2
                dma = nc.sync.dma_start if fast else nc.gpsimd.dma_start
                e = T // HD
                r64 = small1.tile([HD, 2 * e], bf16, tag="r64", name=f"r64_{g}")
                with nc.allow_low_precision("softmax recips in bf16 (tol 2e-2)"):
                    nc.vector.reciprocal(r64, s64_pair)
                for h, yu, half in ((h_on, yu_on, 0), (h_def, yu_def, 1)):
                    scr2 = dramp.tile([T], bf16, tag="scr2", name=f"scr2_{h}")
                    dma(
                        scr2.rearrange("(p e) -> p e", p=HD),
                        r64[:, half * e : (half + 1) * e],
                    )
                    r_sb = small.tile([HD, T], bf16, tag="r", name=f"r{h}")
                    dma(r_sb, scr2[None, :].to_broadcast((HD, T)))
                    if h % 2 == 0:
                        nc.vector.tensor_tensor(
                            yt_sb[g][0:HD, :], yu[0:HD, :], r_sb, ALU.mult
                        )
                    else:
                        ytmp = small1.tile([HD, T], bf16, tag="ytmp", name=f"ytmp{h}")
                        nc.vector.tensor_tensor(ytmp, yu[0:HD, :], r_sb, ALU.mult)
                        # partition shift 0..63 -> 64..127 via SBUF-SBUF DMA
                        dma(yt_sb[g][HD:P, :], ytmp)

            if True:
                pending = []
                for g in range(NT):
                    # pair 7 swaps roles so the no-shift (even) head lands last
                    swap = g == NT - 1
                    h_on, h_def = (2 * g + 1, 2 * g) if swap else (2 * g, 2 * g + 1)
                    m = g
                    if g > 0:
                        emit_qk(m)
                        emit_qk((C // P) + m)
                    if g == 2:
                        # wproj load emitted early enough to overlap attention
                        for s in range(NS):
                            nc.sync.dma_start(wproj_sb[:, s, :], wp_r[:, s, :])
                    sl_on = (HD, P) if swap else (0, HD)
                    sl_def = (0, HD) if swap else (HD, P)
                    qt_on = qkt_sb[sl_on[0] : sl_on[1], m, :]
                    kt_on = qkt_sb[sl_on[0] : sl_on[1], (C // P) + m, :]
                    qt_def = qkt_sb[sl_def[0] : sl_def[1], m, :]
                    kt_def = qkt_sb[sl_def[0] : sl_def[1], (C // P) + m, :]
                    tp_on = (sl_on[0], 0)
                    tp_def = (sl_def[0], 0)
                    yp = pyp.tile([P, T], f32, tag="py", name=f"yp{h_on}")
                    ypA, ypB = yp[:, 0:512], yp[:, 512:T]
                    pt_defs = []
                    yu_on = None
                    for kt in range(NT):
                        if g == 0:
                            emit_v(kt)
                        if kt == 5:
                            yu_on = evac_headA(ypA, h_on)
                        if kt == 2 and pending:
                            norm_pair(pending.pop(0))
                        q0 = kt * P
                        sp_on = psp.tile([P, T], f32, tag="ps", name=f"spA_{g}_{kt}")
                        sp_def = psp.tile([P, T], f32, tag="ps", name=f"spB_{g}_{kt}")
                        if kt <= 3:
                            spans = [(q0, 512), (512, T)]
                        else:
                            spans = [(q0, T)]
                        for lo, hi in spans:
                            nc.tensor.matmul(
                                sp_on[:, lo:hi],
                                kt_on[:, q0 : q0 + P],
                                qt_on[:, lo:hi],
                                start=True,
                                stop=True,
                                tile_position=tp_on,
                            )
                            nc.tensor.matmul(
                                sp_def[:, lo:hi],
                                kt_def[:, q0 : q0 + P],
                                qt_def[:, lo:hi],
                                start=True,
                                stop=True,
                                tile_position=tp_def,
                            )
                        pt_on = ptp.tile([P, T], bf16, tag="pt", name=f"ptA_{g}_{kt}")
                        nc.scalar.activation(
                            pt_on[:, q0:T], sp_on[:, q0:T], AF.Exp, scale=0.125
                        )
                        pt_def = small1.tile(
                            [P, T - q0], bf16, tag=f"ptB_{kt}", name=f"ptB_{g}_{kt}"
                        )
                        nc.scalar.activation(pt_def, sp_def[:, q0:T], AF.Exp, scale=0.125)
                        # mask the diagonal block (k > q within the block -> 0)
                        nc.vector.tensor_tensor(
                            pt_on[:, q0 : q0 + P], pt_on[:, q0 : q0 + P], cmask, ALU.mult
                        )
                        # deferred head's mask on gpsimd (slack before its
                        # AV burst; keeps the DVE off the critical path)
                        nc.gpsimd.affine_select(
                            out=pt_def[:, 0:P],
                            in_=pt_def[:, 0:P],
                            compare_op=ALU.is_ge,
                            fill=0.0,
                            base=0,
                            pattern=[[1, P]],
                            channel_multiplier=-1,
                        )
                        av_matmuls(ypA, ypB, pt_on, h_on * W, kt, 0)
                        pt_defs.append(pt_def)
                    fast = g >= NT - 2
                    s64_pair = small1.tile(
                        [HD, 2 * (T // HD)], bf16, tag="s64", name=f"s64_{g}"
                    )
                    evac_headB(yu_on, ypB, h_on, s64_pair, 0, fast)
                    yp1 = pyp.tile([P, T], f32, tag="py", name=f"yp{h_def}")
                    yp1A, yp1B = yp1[:, 0:512], yp1[:, 512:T]
                    yu_def = None
                    for kt in range(NT):
                        av_matmuls(yp1A, yp1B, pt_defs[kt], h_def * W, kt, kt * P)
                        if kt == 4:
                            yu_def = evac_headA(yp1A, h_def)
                    evac_headB(yu_def, yp1B, h_def, s64_pair, 1, fast)
                    pending.append((yu_on, h_on, yu_def, h_def, s64_pair, fast))

                while pending:
                    norm_pair(pending.pop(0))

            # ---------------- output projection ----------------
            # reuse the V bias tile for the proj bias (V phase is done)
            bias_o = bias_v
            nc.gpsimd.dma_start(bias_o, bproj_d[:][None, :].to_broadcast((P, C)))
            out_r = out_d[:, :].rearrange("(i p) j -> p i j", p=P)
            if True:
                for i in range(NT):
                    ps0 = pmm.tile([P, 512], f32, tag="pmm", name=f"proj{i}_0")
                    ps1 = pmm.tile([P, 512], f32, tag="pmm", name=f"proj{i}_1")
                    for g in range(NT):
                        for ch, ps in ((0, ps0), (1, ps1)):
                            nc.tensor.matmul(
                                ps,
                                yt_sb[g][:, i * P : (i + 1) * P],
                                wproj_sb[:, g, ch * 512 : (ch + 1) * 512],
                                start=(g == 0),
                                stop=(g == NT - 1),
                            )
                    for ch, ps in ((0, ps0), (1, ps1)):
                        ot = outp.tile([P, 512], f32, tag="out")
                        nc.vector.tensor_tensor(
                            ot, ps, bias_o[:, ch * 512 : (ch + 1) * 512], ALU.add
                        )
                        nc.sync.dma_start(out_r[:, i, ch * 512 : (ch + 1) * 512], ot)

    nc.compile()
    return nc


_NC = None


def _get_nc():
    global _NC
    if _NC is None:
        _NC = _build()
    return _NC


def _in_maps(x, Wqkv, bqkv, Wproj, bproj):
    bf = ml_dtypes.bfloat16
    x = np.ascontiguousarray(np.asarray(x, dtype=np.float32).astype(bf))
    shared = {
        "wqkv": np.ascontiguousarray(np.asarray(Wqkv, dtype=np.float32).astype(bf)),
        "bqkv": np.ascontiguousarray(np.asarray(bqkv, dtype=np.float32)),
        "wproj": np.ascontiguousarray(np.asarray(Wproj, dtype=np.float32).astype(bf)),
        "bproj": np.ascontiguousarray(np.asarray(bproj, dtype=np.float32)),
    }
    return [{"x": np.ascontiguousarray(x[b]), **shared} for b in range(B)]


def run(x, Wqkv, bqkv, Wproj, bproj, **run_kwargs):
    """Run on 8 cores; returns (output [B,T,C] fp32, BassKernelResults)."""
    nc = _get_nc()
    res = run_bass_kernel_spmd(
        nc, _in_maps(x, Wqkv, bqkv, Wproj, bproj), core_ids=list(range(B)), **run_kwargs
    )
    out = np.stack([res.results[b]["out"] for b in range(B)]).astype(np.float32)
    return out, res


def kernel(x, Wqkv, bqkv, Wproj, bproj, n_head=None, **_ignored):
    out, _ = run(x, Wqkv, bqkv, Wproj, bproj)
    return out


# revision 30
# speedup vs baseline: 1.0093x; 1.0093x over previous
"""Causal self-attention (B=8, T=1024, C=1024, H=16) on 8 TRN2 NeuronCores.

Sharding: pure data-parallel over batch - core b computes batch element b
with fully replicated weights (B == n_cores, so no collectives needed).

v2 vs baseline:
  - x / Wqkv / Wproj are cast to bf16 on the HOST, so no on-device weight
    casts and half the weight DMA traffic.
  - x is transposed by the DMA xbar (dma_start_transpose) straight from
    DRAM into SBUF - no PE transposes, no PSUM evacuation copies.
  - QKV / V / proj matmul loops are ordered so consecutive matmuls share
    the stationary operand (halves LDWEIGHTS traffic).
  - Score matmuls for the head pair are emitted interleaved with explicit
    tile_position (0,0)/(64,0) so the PE can run both 64-contraction
    matmuls concurrently in different row groups.
  - Y evacuation split across ACT+DVE; V/proj-bias evacuations on Pool;
    softmax reciprocal chain batched per head with a DMA reblock; the
    last pair uses a DVE-recip + PE-broadcast fast path to cut the tail.
"""

import numpy as np
import ml_dtypes

import concourse.tile as tile
from concourse import bacc, mybir
from concourse.bass_utils import run_bass_kernel_spmd
from concourse.masks import make_identity

f32 = mybir.dt.float32
bf16 = mybir.dt.bfloat16
AF = mybir.ActivationFunctionType
ALU = mybir.AluOpType

B, T, C, H, HD = 8, 1024, 1024, 16, 64
P = 128
NT = T // P  # 8 token tiles
NS = C // P  # 8 contraction subtiles
W = 66  # per-head stride in V_sb: [64 vals][1 ones][1 pad]


def _build():
    nc = bacc.Bacc(trn_type="TRN2")
    x_d = nc.dram_tensor("x", (T, C), bf16, kind="ExternalInput")
    wqkv_d = nc.dram_tensor("wqkv", (C, 3 * C), bf16, kind="ExternalInput")
    bqkv_d = nc.dram_tensor("bqkv", (3 * C,), f32, kind="ExternalInput")
    # host-prepared column-major view of bqkv[:2C]: bqk_cm[p, o] = bqkv[o*128+p]
    bqk_cm_d = nc.dram_tensor("bqk_cm", (P, 2 * C // P), f32, kind="ExternalInput")
    wproj_d = nc.dram_tensor("wproj", (C, C), bf16, kind="ExternalInput")
    bproj_d = nc.dram_tensor("bproj", (C,), f32, kind="ExternalInput")
    out_d = nc.dram_tensor("out", (T, C), f32, kind="ExternalOutput")

    with tile.TileContext(nc) as tc:
        with (
            tc.tile_pool(name="big", bufs=1) as big,
            tc.tile_pool(name="ptp", bufs=4) as ptp,
            tc.tile_pool(name="small", bufs=3) as small,
            tc.tile_pool(name="small1", bufs=1) as small1,
            tc.tile_pool(name="outp", bufs=3) as outp,
            tc.tile_pool(name="dramp", bufs=6, space="DRAM") as dramp,
            tc.tile_pool(name="pmm", bufs=2, space="PSUM") as pmm,
            tc.tile_pool(name="psp", bufs=2, space="PSUM") as psp,
            tc.tile_pool(name="pyp", bufs=1, space="PSUM") as pyp,
        ):
            # ---------------- x transpose via DMA xbar, first on sync -----
            # xt[p, s, t] = x[t, s*128+p]; two c-halves so the QKV
            # contraction can start on s 0..3 while the second half streams.
            xt_sb = big.tile([P, NS, T], bf16, tag="xt")
            nc.sync.dma_start_transpose(xt_sb[:, 0 : NS // 2, :], x_d[:, 0 : C // 2])
            nc.sync.dma_start_transpose(xt_sb[:, NS // 2 :, :], x_d[:, C // 2 :])

            # ---------------- weights (bf16, no casts) ----------------
            # Q/K m-pair slices on the ACT ring so they overlap the x
            # transpose on the sync ring; ACT's ring is idle at startup
            # (first exp is ~15us in) and these issues finish well before.
            # Ordered mp 0,4,1,5,... so each pair's Q (mp<4) and K (mp>=4)
            # slices arrive together.
            wqkv_sb = big.tile([P, NS, 3 * C], bf16, tag="wqkv")
            wq_r = wqkv_d[:, :].rearrange("(s p) i -> p s i", p=P)
            for mp in (0, 4, 1, 5, 2, 6, 3, 7):
                nc.scalar.dma_start(
                    wqkv_sb[:, :, mp * 256 : (mp + 1) * 256],
                    wq_r[:, :, mp * 256 : (mp + 1) * 256],
                )

            # V columns per s-row (2KB descriptors): sync ring, so they
            # queue behind the x transpose and don't steal its bandwidth
            for s in range(NS):
                nc.sync.dma_start(
                    wqkv_sb[:, s, 2 * C : 3 * C], wq_r[:, s, 2 * C : 3 * C]
                )

            # ---------------- constants ----------------
            # causal multiplicative mask for the transposed diagonal block:
            # cmask[k, q] = 1 if q >= k else 0
            cmask = big.tile([P, P], bf16, tag="cmask")
            nc.gpsimd.memset(cmask, 1.0)
            nc.gpsimd.affine_select(
                out=cmask,
                in_=cmask,
                compare_op=ALU.is_ge,
                fill=0.0,
                base=0,
                pattern=[[1, P]],
                channel_multiplier=-1,
            )
            # per-partition bias columns for the Q/K part of qkvT
            # (host-prepared layout: one 64B descriptor per partition)
            bqk_col = big.tile([P, 2 * C // P], f32, tag="bqk")
            nc.gpsimd.dma_start(bqk_col, bqk_cm_d[:, :])
            # broadcast bias rows (per free-dim column) for V and proj
            bias_v = big.tile([P, C], f32, tag="bias_v")
            nc.gpsimd.dma_start(bias_v, bqkv_d[2 * C :][None, :].to_broadcast((P, C)))
            wproj_sb = big.tile([P, NS, C], bf16, tag="wproj")
            wp_r = wproj_d[:, :].rearrange("(s p) j -> p s j", p=P)

            # ---------------- Q/K^T tiles (interleaved with attention) ----
            qkt_sb = big.tile([P, 2 * C // P, T], bf16, tag="qkt")

            def emit_qk(m):
                # ch-inner so both matmuls share one LDWEIGHTS per s
                ps0 = pmm.tile([P, 512], f32, tag="pmm", name=f"qk{m}_0")
                ps1 = pmm.tile([P, 512], f32, tag="pmm", name=f"qk{m}_1")
                for s in range(NS):
                    for ch, ps in ((0, ps0), (1, ps1)):
                        nc.tensor.matmul(
                            ps,
                            wqkv_sb[:, s, m * P : (m + 1) * P],
                            xt_sb[:, s, ch * 512 : (ch + 1) * 512],
                            start=(s == 0),
                            stop=(s == NS - 1),
                        )
                nc.vector.tensor_scalar_add(
                    qkt_sb[:, m, 0:512], ps0, bqk_col[:, m : m + 1]
                )
                nc.vector.tensor_scalar_add(
                    qkt_sb[:, m, 512:T], ps1, bqk_col[:, m : m + 1]
                )

            # pair 0's Q/K first so ScalarE's exp pipeline starts early
            emit_qk(0)
            emit_qk(C // P)

            # ---------------- V (natural layout, ones-augmented) ----------
            v_sb = [
                big.tile([P, H * W], bf16, tag=f"v{i}", name=f"v{i}") for i in range(NT)
            ]

            def emit_v(i):
                v3 = v_sb[i].rearrange("p (h w) -> p h w", w=W)
                nc.gpsimd.memset(v3[:, :, HD : HD + 1], 1.0)
                ps0 = pmm.tile([P, 512], f32, tag="pmm", name=f"v{i}_0")
                ps1 = pmm.tile([P, 512], f32, tag="pmm", name=f"v{i}_1")
                for s in range(NS):
                    for ch, ps in ((0, ps0), (1, ps1)):
                        nc.tensor.matmul(
                            ps,
                            xt_sb[:, s, i * P : (i + 1) * P],
                            wqkv_sb[:, s, 2 * C + ch * 512 : 2 * C + (ch + 1) * 512],
                            start=(s == 0),
                            stop=(s == NS - 1),
                        )
                for ch, ps in ((0, ps0), (1, ps1)):
                    nc.vector.tensor_tensor(
                        v3[:, 8 * ch : 8 * ch + 8, 0:HD],
                        ps.rearrange("p (h d) -> p h d", d=HD),
                        bias_v[:, ch * 512 : (ch + 1) * 512].rearrange(
                            "p (h d) -> p h d", d=HD
                        ),
                        ALU.add,
                    )

            # ---------------- attention ----------------
            yt_sb = [
                big.tile([P, T], bf16, tag=f"yt{g}", name=f"yt{g}") for g in range(NT)
            ]

            def s_matmuls(sp0, sp1, kt_h0, qt_h0, kt_h1, qt_h1, kt):
                # interleave the two heads' matmuls so they occupy
                # different PE row groups concurrently
                q0 = kt * P
                if kt <= 3:
                    spans = [(q0, 512), (512, T)]
                else:
                    spans = [(q0, T)]
                for lo, hi in spans:
                    nc.tensor.matmul(
                        sp0[:, lo:hi],
                        kt_h0[:, q0 : q0 + P],
                        qt_h0[:, lo:hi],
                        start=True,
                        stop=True,
                        tile_position=(0, 0),
                    )
                    nc.tensor.matmul(
                        sp1[:, lo:hi],
                        kt_h1[:, q0 : q0 + P],
                        qt_h1[:, lo:hi],
                        start=True,
                        stop=True,
                        tile_position=(64, 0),
                    )

            def av_matmuls(ypA, ypB, pt_ap, vcols, kt, q_off):
                # ypA covers q columns [0,512), ypB [512,T); pt_ap covers
                # q columns [q_off, T); accumulate over kt
                q0 = kt * P
                lhsT_v = v_sb[kt][:, vcols : vcols + HD + 1]  # [128, 65]
                if kt <= 3:
                    nc.tensor.matmul(
                        ypA[0 : HD + 1, q0:512],
                        lhsT_v,
                        pt_ap[:, q0 - q_off : 512 - q_off],
                        start=(kt == 0),
                        stop=(kt == 3),
                    )
                    nc.tensor.matmul(
                        ypB[0 : HD + 1, 0:512],
                        lhsT_v,
                        pt_ap[:, 512 - q_off : T - q_off],
                        start=(kt == 0),
                        stop=(kt == NT - 1),
                    )
                else:
                    nc.tensor.matmul(
                        ypB[0 : HD + 1, q0 - 512 : 512],
                        lhsT_v,
                        pt_ap[:, q0 - q_off : T - q_off],
                        start=False,
                        stop=(kt == NT - 1),
                    )

            def evac_headA(ypA, h):
                # the A half (q cols 0:512) finishes accumulating at kt=3,
                # so it is evacuated early, off the critical path
                yu = small.tile([HD + 1, T], bf16, tag="yu", name=f"yu{h}")
                nc.scalar.copy(yu[:, 0:512], ypA[0 : HD + 1, 0:512])
                return yu

            def evac_headB(yu, ypB, h, s64_pair, half, fast):
                # the B half gates the psum recycle: split ACT/DVE quarters
                nc.scalar.copy(yu[:, 512:768], ypB[0 : HD + 1, 0:256])
                nc.vector.tensor_copy(yu[:, 768:T], ypB[0 : HD + 1, 256:512])
                # start the reciprocal DMA chain (reblock via DRAM); both
                # heads of a pair land in one s64 tile so one reciprocal
                # serves the pair
                dma = nc.sync.dma_start if fast else nc.gpsimd.dma_start
                scr = dramp.tile([T], bf16, tag="scr", name=f"scr{h}")
                dma(scr[None, :], yu[HD : HD + 1, :])
                e = T // HD
                dma(
                    s64_pair[:, half * e : (half + 1) * e],
                    scr.rearrange("(p e) -> p e", p=HD),
                )

            def _norm_one(h, yu, r64, half, dma):
                # broadcast this head's reciprocal row + normalize into yt
                g = h // 2
                e = T // HD
                scr2 = dramp.tile([T], bf16, tag="scr2", name=f"scr2_{h}")
                dma(
                    scr2.rearrange("(p e) -> p e", p=HD),
                    r64[:, half * e : (half + 1) * e],
                )
                r_sb = small.tile([HD, T], bf16, tag="r", name=f"r{h}")
                dma(r_sb, scr2[None, :].to_broadcast((HD, T)))
                if h % 2 == 0:
                    nc.vector.tensor_tensor(yt_sb[g][0:HD, :], yu[0:HD, :], r_sb, ALU.mult)
                else:
                    ytmp = small1.tile([HD, T], bf16, tag="ytmp", name=f"ytmp{h}")
                    nc.vector.tensor_tensor(ytmp, yu[0:HD, :], r_sb, ALU.mult)
                    # partition shift 0..63 -> 64..127 via SBUF-SBUF DMA
                    dma(yt_sb[g][HD:P, :], ytmp)

            def norm_entry(state):
                # reciprocal + broadcast + normalize; emitted ~a pair later
                # so the DVE never head-of-line blocks on the DMA chain
                if state[0] == "pair":
                    _, yu_on, h_on, yu_def, h_def, s64_pair = state
                    r64 = small1.tile(
                        [HD, 2 * (T // HD)], bf16, tag="r64", name=f"r64_{h_on // 2}"
                    )
                    with nc.allow_low_precision("softmax recips in bf16 (tol 2e-2)"):
                        nc.vector.reciprocal(r64, s64_pair)
                    dma = nc.gpsimd.dma_start
                    _norm_one(h_on, yu_on, r64, 0, dma)
                    _norm_one(h_def, yu_def, r64, 1, dma)
                else:
                    # single-head fast chain for the last pairs (sync ring)
                    _, yu, h, s64 = state
                    r64 = small1.tile(
                        [HD, T // HD], bf16, tag=f"r64f{h}", name=f"r64f{h}"
                    )
                    with nc.allow_low_precision("softmax recips in bf16 (tol 2e-2)"):
                        nc.vector.reciprocal(r64, s64)
                    _norm_one(h, yu, r64, 0, nc.sync.dma_start)

            if True:
                pending = []
                for g in range(NT):
                    # pair 7 swaps roles so the no-shift (even) head lands last
                    swap = g == NT - 1
                    h_on, h_def = (2 * g + 1, 2 * g) if swap else (2 * g, 2 * g + 1)
                    m = g
                    if g > 0:
                        emit_qk(m)
                        emit_qk((C // P) + m)
                    if g == 2:
                        # wproj load emitted early enough to overlap attention
                        for s in range(NS):
                            nc.sync.dma_start(wproj_sb[:, s, :], wp_r[:, s, :])
                    sl_on = (HD, P) if swap else (0, HD)
                    sl_def = (0, HD) if swap else (HD, P)
                    qt_on = qkt_sb[sl_on[0] : sl_on[1], m, :]
                    kt_on = qkt_sb[sl_on[0] : sl_on[1], (C // P) + m, :]
                    qt_def = qkt_sb[sl_def[0] : sl_def[1], m, :]
                    kt_def = qkt_sb[sl_def[0] : sl_def[1], (C // P) + m, :]
                    tp_on = (sl_on[0], 0)
                    tp_def = (sl_def[0], 0)
                    yp = pyp.tile([P, T], f32, tag="py", name=f"yp{h_on}")
                    ypA, ypB = yp[:, 0:512], yp[:, 512:T]
                    pt_defs = []
                    yu_on = None
                    for kt in range(NT):
                        if g == 0:
                            emit_v(kt)
                        if kt == 5:
                            yu_on = evac_headA(ypA, h_on)
                        if kt in (2, 5) and pending:
                            norm_entry(pending.pop(0))
                        q0 = kt * P
                        sp_on = psp.tile([P, T], f32, tag="ps", name=f"spA_{g}_{kt}")
                        sp_def = psp.tile([P, T], f32, tag="ps", name=f"spB_{g}_{kt}")
                        if kt <= 3:
                            spans = [(q0, 512), (512, T)]
                        else:
                            spans = [(q0, T)]
                        for lo, hi in spans:
                            nc.tensor.matmul(
                                sp_on[:, lo:hi],
                                kt_on[:, q0 : q0 + P],
                                qt_on[:, lo:hi],
                                start=True,
                                stop=True,
                                tile_position=tp_on,
                            )
                            nc.tensor.matmul(
                                sp_def[:, lo:hi],
                                kt_def[:, q0 : q0 + P],
                                qt_def[:, lo:hi],
                                start=True,
                                stop=True,
                                tile_position=tp_def,
                            )
                        pt_on = ptp.tile([P, T], bf16, tag="pt", name=f"ptA_{g}_{kt}")
                        nc.scalar.activation(
                            pt_on[:, q0:T], sp_on[:, q0:T], AF.Exp, scale=0.125
                        )
                        pt_def = small1.tile(
                            [P, T - q0], bf16, tag=f"ptB_{kt}", name=f"ptB_{g}_{kt}"
                        )
                        nc.scalar.activation(pt_def, sp_def[:, q0:T], AF.Exp, scale=0.125)
                        # mask the diagonal block (k > q within the block -> 0)
                        nc.vector.tensor_tensor(
                            pt_on[:, q0 : q0 + P], pt_on[:, q0 : q0 + P], cmask, ALU.mult
                        )
                        # deferred head's mask on gpsimd (slack before its
                        # AV burst; keeps the DVE off the critical path)
                        nc.gpsimd.affine_select(
                            out=pt_def[:, 0:P],
                            in_=pt_def[:, 0:P],
                            compare_op=ALU.is_ge,
                            fill=0.0,
                            base=0,
                            pattern=[[1, P]],
                            channel_multiplier=-1,
                        )
                        av_matmuls(ypA, ypB, pt_on, h_on * W, kt, 0)
                        pt_defs.append(pt_def)
                    fast = g >= NT - 2
                    if fast:
                        s64_on = small1.tile(
                            [HD, T // HD], bf16, tag=f"s64f{h_on}", name=f"s64f{h_on}"
                        )
                        evac_headB(yu_on, ypB, h_on, s64_on, 0, True)
                        pending.append(("single", yu_on, h_on, s64_on))
                    else:
                        s64_pair = small1.tile(
                            [HD, 2 * (T // HD)], bf16, tag="s64", name=f"s64_{g}"
                        )
                        evac_headB(yu_on, ypB, h_on, s64_pair, 0, False)
                    yp1 = pyp.tile([P, T], f32, tag="py", name=f"yp{h_def}")
                    yp1A, yp1B = yp1[:, 0:512], yp1[:, 512:T]
                    yu_def = None
                    for kt in range(NT):
                        av_matmuls(yp1A, yp1B, pt_defs[kt], h_def * W, kt, kt * P)
                        if kt == 4:
                            yu_def = evac_headA(yp1A, h_def)
                    if fast:
                        s64_def = small1.tile(
                            [HD, T // HD], bf16, tag=f"s64f{h_def}", name=f"s64f{h_def}"
                        )
                        evac_headB(yu_def, yp1B, h_def, s64_def, 0, True)
                        pending.append(("single", yu_def, h_def, s64_def))
                    else:
                        evac_headB(yu_def, yp1B, h_def, s64_pair, 1, False)
                        pending.append(
                            ("pair", yu_on, h_on, yu_def, h_def, s64_pair)
                        )

                while pending:
                    norm_entry(pending.pop(0))

            # ---------------- output projection ----------------
            # reuse the V bias tile for the proj bias (V phase is done)
            bias_o = bias_v
            nc.gpsimd.dma_start(bias_o, bproj_d[:][None, :].to_broadcast((P, C)))
            out_r = out_d[:, :].rearrange("(i p) j -> p i j", p=P)
            # i=0,1 accumulate g=0..6 first so the PE has guaranteed work
            # while the last pair's normalization chain completes; their g=7
            # contribution + everything else follows.
            def proj_mms(i, ps0, ps1, gs):
                for g in gs:
                    for ps, ch in ((ps0, 0), (ps1, 1)):
                        nc.tensor.matmul(
                            ps,
                            yt_sb[g][:, i * P : (i + 1) * P],
                            wproj_sb[:, g, ch * 512 : (ch + 1) * 512],
                            start=(g == 0),
                            stop=(g == NT - 1),
                        )

            def proj_evac(i, ps0, ps1):
                for ps, ch in ((ps0, 0), (ps1, 1)):
                    ot = outp.tile([P, 512], f32, tag="out")
                    nc.vector.tensor_tensor(
                        ot, ps, bias_o[:, ch * 512 : (ch + 1) * 512], ALU.add
                    )
                    nc.sync.dma_start(out_r[:, i, ch * 512 : (ch + 1) * 512], ot)

            p00 = pmm.tile([P, 512], f32, tag="pmm", name="proj0_0")
            p01 = pmm.tile([P, 512], f32, tag="pmm", name="proj0_1")
            proj_mms(0, p00, p01, range(NT - 1))
            for i in range(NT):
                if i == 0:
                    ps0, ps1 = p00, p01
                    proj_mms(i, ps0, ps1, [NT - 1])
                else:
                    ps0 = pmm.tile([P, 512], f32, tag="pmm", name=f"proj{i}_0")
                    ps1 = pmm.tile([P, 512], f32, tag="pmm", name=f"proj{i}_1")
                    proj_mms(i, ps0, ps1, range(NT))
                proj_evac(i, ps0, ps1)

    nc.compile()
    return nc


_NC = None


def _get_nc():
    global _NC
    if _NC is None:
        _NC = _build()
    return _NC


def _in_maps(x, Wqkv, bqkv, Wproj, bproj):
    bf = ml_dtypes.bfloat16
    x = np.ascontiguousarray(np.asarray(x, dtype=np.float32).astype(bf))
    bqkv = np.asarray(bqkv, dtype=np.float32)
    shared = {
        "wqkv": np.ascontiguousarray(np.asarray(Wqkv, dtype=np.float32).astype(bf)),
        "bqkv": np.ascontiguousarray(bqkv),
        "bqk_cm": np.ascontiguousarray(bqkv[: 2 * C].reshape(2 * C // P, P).T),
        "wproj": np.ascontiguousarray(np.asarray(Wproj, dtype=np.float32).astype(bf)),
        "bproj": np.ascontiguousarray(np.asarray(bproj, dtype=np.float32)),
    }
    return [{"x": np.ascontiguousarray(x[b]), **shared} for b in range(B)]


def run(x, Wqkv, bqkv, Wproj, bproj, **run_kwargs):
    """Run on 8 cores; returns (output [B,T,C] fp32, BassKernelResults)."""
    nc = _get_nc()
    res = run_bass_kernel_spmd(
        nc, _in_maps(x, Wqkv, bqkv, Wproj, bproj), core_ids=list(range(B)), **run_kwargs
    )
    out = np.stack([res.results[b]["out"] for b in range(B)]).astype(np.float32)
    return out, res


def kernel(x, Wqkv, bqkv, Wproj, bproj, n_head=None, **_ignored):
    out, _ = run(x, Wqkv, bqkv, Wproj, bproj)
    return out
